# revision 1
# baseline (speedup 1.0000x reference)
"""Strided depthwise-conv ("CompressKV") kernel for 8 Trainium2 NeuronCores.

y[b,m,h,d] = (sum_k x[b, 16*m+k, h, d] * w[k] + sum_k pe[k,d]*w[k]) / 32
B=4, N=16384, H=8, D=128, K=32, STRIDE=16, M=1023.

Strategy
--------
Shard: core <-> (batch b, sequence half). Each core owns one contiguous
token slab x[b, 8192*hh : 8192*hh+8320] (zero-padded past N), all 8 heads.

Compute: the strided conv is expressed as 17 banded-weight matmuls per
128-m output tile on the TensorEngine:

    y[m', f] = sum_i  W_i[n', m'].T @ X_chunk[16*ot+i][n', f]

where chunk g = tokens [128g, 128g+128), f = (head, d) flattened (1024),
W_i[n', m'] = w[128*i + n' - 16*m'] / 32  (zero outside [0,32)).
W_i is built on the host from `weight` and fed as a small extra input.
The pe-bias vector enters the same PSUM accumulation as a rank-2 matmul
(ones.T @ [bias_hi; bias_lo], hi/lo bf16 split keeps it fp32-exact).
x is cast to bf16 on the host (halves DMA bytes; matmul runs at bf16
rate).  PSUM bank limit (512 fp32) => each logical matmul is two
512-wide matmuls.  Eviction is a scalar-engine copy + scalar-issued
store so no DMA instruction ever needs more than one semaphore wait
(walrus DIRECT2D limit).
"""

import numpy as np
import ml_dtypes
from contextlib import ExitStack

import concourse.bass as bass
import concourse.mybir as mybir
import concourse.tile as tile
from concourse.bass import ds, ts
from concourse.bass_utils import run_bass_kernel_spmd

BF16 = ml_dtypes.bfloat16


class _SplitDrainTileContext(tile.TileContext):
    """TileContext whose kernel-tail drain carries at most one sem wait.

    TRN2 instructions have a single sync-wait slot; the stock tail drain
    aggregates one wait per logical processor (14 here), which walrus
    rejects.  Move the extras onto dedicated single-wait nops on the same
    (sync) queue ahead of the all-engine barrier — identical semantics.
    """

    def _drain_and_barrier(self, tick_clock, wait_clock):
        import bass_rust
        from concourse.vector_clock import ScopedClock

        drain_inst = self.nc.sync.drain()
        wait_clock.add_sem_waits(
            drain_inst.ins, ScopedClock({None: tick_clock.global_clock}))
        si = drain_inst.ins.sync_info
        if si is not None and len(si.on_wait) > 1:
            waits = list(si.on_wait)
            drain_inst.ins.sync_info = bass_rust.SyncInfo(
                on_wait=[waits[0]], on_update=list(si.on_update))
            for w in waits[1:]:
                nop = self.nc.sync.nop(hint="drain_split", nofuse=True)
                nop.ins.sync_info = bass_rust.SyncInfo(
                    on_wait=[w], on_update=[])

        self.nc.all_engine_barrier()
        assert self.sems is not None
        popped = self.nc._tile_sem_poison_stack.pop()
        assert popped is self._sem_poison
        self.nc.clear_and_free_semaphores(
            list(self.sems.allocated().values()))
        self.nc.all_engine_barrier()

B, N, H, D = 4, 16384, 8, 128
KS, STRIDE = 32, 16
M = (N - KS) // STRIDE + 1      # 1023
NCORES = 8
F = H * D                        # 1024 free elems (head, d)
P = 128                          # partitions / tokens per chunk
NW = 17                          # band matrices per output tile
CH = 65                          # chunks per core slab (8320 tokens)
OT = 4                           # output tiles of 128 m per core
T_SLAB = CH * P                  # 8320
HF = F // 2                      # 512 = one PSUM bank of fp32
PC = 4                           # chunks per load DMA (1 MiB transfers)

_prog_cache = {}


def _split_multi_waits(nc):
    """TRN2 instructions carry one sync-wait slot; Tile sometimes attaches
    more (slot-recycle + DMA-lane).  Hoist extras onto single-wait nops
    inserted just before the instruction on the same engine queue —
    identical semantics, accepted by walrus codegen."""
    import bass_rust
    for func in nc.m.functions:
        for bb in func.blocks:
            insts = list(bb.instructions)
            out, changed = [], False
            for inst in insts:
                si = inst.sync_info
                if si is not None and len(si.on_wait) > 1:
                    waits = list(si.on_wait)
                    for k, w in enumerate(waits[:-1]):
                        nop = mybir.InstNoOp(name=f"{inst.name}-ws{k}")
                        nop.engine = inst.engine
                        nop.sync_info = bass_rust.SyncInfo(
                            on_wait=[w], on_update=[])
                        out.append(nop)
                    inst.sync_info = bass_rust.SyncInfo(
                        on_wait=[waits[-1]], on_update=list(si.on_update))
                    changed = True
                out.append(inst)
            if changed:
                bb.instructions = out


def _build_program(reps=1):
    """Build the SPMD Bass/Tile program (identical for all 8 cores).

    reps>1 repeats the whole pipeline inside one NEFF (benchmark use:
    slope of wall time vs reps isolates device execution time from the
    dispatch round trip)."""
    nc = bass.Bass("TRN2", target_bir_lowering=False, debug=False,
                   num_devices=NCORES)
    x_d = nc.dram_tensor("x", [T_SLAB, F], mybir.dt.bfloat16,
                         kind="ExternalInput").ap()
    w_d = nc.dram_tensor("wt", [P, NW * P], mybir.dt.bfloat16,
                         kind="ExternalInput").ap()
    br_d = nc.dram_tensor("brow", [2, F], mybir.dt.bfloat16,
                          kind="ExternalInput").ap()
    on_d = nc.dram_tensor("ones2", [2, P], mybir.dt.bfloat16,
                          kind="ExternalInput").ap()
    y_d = nc.dram_tensor("y", [OT * P, F], mybir.dt.float32,
                         kind="ExternalOutput").ap()

    with _SplitDrainTileContext(nc) as tc, ExitStack() as ctx:
        const_pool = ctx.enter_context(tc.tile_pool(name="const", bufs=1))
        chunk_pool = ctx.enter_context(
            tc.tile_pool(name="chunks", bufs=CH // PC + 1))
        out_pool = ctx.enter_context(tc.tile_pool(name="out", bufs=OT))
        psum_pool = ctx.enter_context(
            tc.tile_pool(name="psum", bufs=OT, space="PSUM"))

        wt = const_pool.tile([P, NW * P], mybir.dt.bfloat16)
        nc.scalar.dma_start(out=wt[:], in_=w_d)
        brow = const_pool.tile([2, F], mybir.dt.bfloat16)
        nc.scalar.dma_start(out=brow[:], in_=br_d)
        ones2 = const_pool.tile([2, P], mybir.dt.bfloat16)
        nc.scalar.dma_start(out=ones2[:], in_=on_d)

        for _rep in range(reps):
            psum_tiles = {}

            def evict(ot):
                o = out_pool.tile([P, F], mybir.dt.float32, name="o", tag="o")
                nc.vector.tensor_copy(o[:], psum_tiles[ot][:])
                nc.gpsimd.dma_start(out=y_d[ds(ot * P, P)], in_=o[:])

            def process(g, rhs_of):
                ot, i = g // 16, g % 16
                if i == 0 and g > 0:
                    # W_16 closes the previous output tile's accumulation
                    for hf in range(2):
                        nc.tensor.matmul(
                            psum_tiles[ot - 1][:, ts(hf, HF)],
                            lhsT=wt[:, ts(16, P)],
                            rhs=rhs_of(hf),
                            start=False, stop=True)
                    evict(ot - 1)
                if g < 16 * OT:
                    if i == 0:
                        psum_tiles[ot] = psum_pool.tile(
                            [P, F], mybir.dt.float32, name="ps", tag="ps")
                        # bias enters the accumulation as a rank-2 matmul
                        for hf in range(2):
                            nc.tensor.matmul(
                                psum_tiles[ot][:, ts(hf, HF)],
                                lhsT=ones2[:],
                                rhs=brow[:, ts(hf, HF)],
                                start=True, stop=False)
                    for hf in range(2):
                        nc.tensor.matmul(
                            psum_tiles[ot][:, ts(hf, HF)],
                            lhsT=wt[:, ts(i, P)],
                            rhs=rhs_of(hf),
                            start=False, stop=False)

            # PC chunks per DMA (1 MiB): strided DRAM view interleaves
            # chunk-major rows into one [P, PC*F] SBUF tile
            g = 0
            while g < CH:
                pc = min(PC, CH - g)
                grp = chunk_pool.tile([P, pc * F], mybir.dt.bfloat16,
                                      name="grp", tag="chunk")
                if pc > 1:
                    nc.sync.dma_start(
                        out=grp[:].rearrange("p (c f) -> p c f", c=pc),
                        in_=x_d[ds(P * g, P * pc)].rearrange(
                            "(c p) f -> p c f", p=P))
                else:
                    nc.sync.dma_start(out=grp[:], in_=x_d[ds(P * g, P)])
                for c in range(pc):
                    process(g + c,
                            lambda hf, c=c: grp[:, ds(c * F + hf * HF, HF)])
                g += pc
    _split_multi_waits(nc)
    return nc


def _get_program(reps=1):
    if reps not in _prog_cache:
        _prog_cache[reps] = _build_program(reps)
    return _prog_cache[reps]


def _host_prep(x, weight, pe):
    """Build per-core input maps (band matrices, bias rows, bf16 slabs)."""
    x = np.asarray(x)
    weight = np.asarray(weight, dtype=np.float32)
    pe = np.asarray(pe, dtype=np.float32)

    i_ = np.arange(NW)[:, None, None]
    n_ = np.arange(P)[None, :, None]
    m_ = np.arange(P)[None, None, :]
    k_ = 128 * i_ + n_ - 16 * m_
    wt = np.where((k_ >= 0) & (k_ < KS),
                  weight[np.clip(k_, 0, KS - 1)] / KS, 0.0)
    # [NW, n, m] -> [n, NW*m] so the SBUF tile loads with one plain 2D DMA
    wt = wt.astype(BF16).transpose(1, 0, 2).reshape(P, NW * P)

    bias_d = ((weight[:, None].astype(np.float64) * pe).sum(0) / KS
              ).astype(np.float32)
    bias_hi = bias_d.astype(BF16)
    bias_lo = (bias_d - bias_hi.astype(np.float32)).astype(BF16)
    brow = np.stack([np.tile(bias_hi, H), np.tile(bias_lo, H)])  # [2, 1024]
    ones2 = np.ones((2, P), dtype=BF16)

    in_maps = []
    for c in range(NCORES):
        b, hh = c // 2, c % 2
        base = 8192 * hh
        t_valid = min(N - base, T_SLAB)
        slab = np.zeros((T_SLAB, F), dtype=BF16)
        slab[:t_valid] = x[b, base:base + t_valid].reshape(t_valid, F)
        in_maps.append({"x": slab, "wt": wt, "brow": brow, "ones2": ones2})
    return in_maps


def _assemble(results, dtype):
    y = np.empty((B, M, H, D), dtype=np.float32)
    for c in range(NCORES):
        b, hh = c // 2, c % 2
        rows = 512 if hh == 0 else M - 512
        part = results[c]["y"].reshape(OT * P, H, D)
        y[b, 512 * hh:512 * hh + rows] = part[:rows]
    return y.astype(dtype, copy=False)


def kernel(x, weight, pe):
    nc = _get_program()
    in_maps = _host_prep(x, weight, pe)
    res = run_bass_kernel_spmd(nc, in_maps, list(range(NCORES)))
    return _assemble(res.results, np.asarray(x).dtype)



# revision 2
# speedup vs baseline: 1.7211x; 1.7211x over previous
"""Strided depthwise-conv ("CompressKV") kernel for 8 Trainium2 NeuronCores.

y[b,m,h,d] = (sum_k x[b, 16*m+k, h, d] * w[k] + sum_k pe[k,d]*w[k]) / 32
B=4, N=16384, H=8, D=128, K=32, STRIDE=16, M=1023.

Strategy
--------
Shard: core <-> (batch b, sequence half). Each core owns the token slab
x[b, 8192*hh : 8192*hh+8208] (zero-padded past N), all 8 heads.

Compute: x is the STATIONARY matmul operand. Since kernel_size = 2*stride,
each 16-token block j feeds exactly two outputs:

    y[m] = sum_s x_r[m,   s]*w[s]      (A-term, s<16)
         + sum_s x_r[m+1, s]*w[16+s]   (B-term)

so a 128-token chunk c (blocks 8c..8c+7) contributes to the 9 outputs
m = 8c-1 .. 8c+7 with a CONSTANT [128, 9] banded weight matrix wband:
wband[16u+s, p] = w[s]/32 if u==p-1 else w[16+s]/32 if u==p else 0.

Per (chunk, head): one matmul with lhsT = x_chunk[:, head] (fp8 e3m4,
128x128) and rhs = wband (bf16, 128x9), accumulated into PSUM holding
y^T[d, m] for that head. The moving operand is 9 columns wide, so PE time
is negligible; x rides the zero-cost weight-load path. x is cast to
fp8e3m4 on the host (halves DMA vs bf16; measured end-to-end rel err
9.5e-3 vs the 2e-2 budget). The pe-bias enters via a rank-2 ones matmul
(bf16 hi/lo split) that also zero-initializes each PSUM region.

PSUM: one 2KB bank = [128, 512] fp32 holds one m-quarter (128 outputs)
for 4 heads. Quarters evict (fp32->bf16 copy on DVE/Act) as soon as the
next quarter's boundary B-column lands, and each 256-output half stores
with a single DMA (512B-contiguous rows).
"""

import numpy as np
import ml_dtypes
from contextlib import ExitStack

import concourse.bass as bass
import concourse.mybir as mybir
import concourse.tile as tile
from concourse.bass import ds, ts
from concourse.bass_utils import run_bass_kernel_spmd

BF16 = ml_dtypes.bfloat16
E3M4 = ml_dtypes.float8_e3m4


class _SplitDrainTileContext(tile.TileContext):
    """TileContext whose kernel-tail drain carries at most one sem wait.

    TRN2 instructions have a single sync-wait slot; the stock tail drain
    aggregates one wait per logical processor (14 here), which walrus
    rejects.  Move the extras onto dedicated single-wait nops on the same
    (sync) queue ahead of the all-engine barrier — identical semantics.
    """

    def _drain_and_barrier(self, tick_clock, wait_clock):
        import bass_rust
        from concourse.vector_clock import ScopedClock

        drain_inst = self.nc.sync.drain()
        wait_clock.add_sem_waits(
            drain_inst.ins, ScopedClock({None: tick_clock.global_clock}))
        si = drain_inst.ins.sync_info
        if si is not None and len(si.on_wait) > 1:
            waits = list(si.on_wait)
            drain_inst.ins.sync_info = bass_rust.SyncInfo(
                on_wait=[waits[0]], on_update=list(si.on_update))
            for w in waits[1:]:
                nop = self.nc.sync.nop(hint="drain_split", nofuse=True)
                nop.ins.sync_info = bass_rust.SyncInfo(
                    on_wait=[w], on_update=[])

        self.nc.all_engine_barrier()
        assert self.sems is not None
        popped = self.nc._tile_sem_poison_stack.pop()
        assert popped is self._sem_poison
        self.nc.clear_and_free_semaphores(
            list(self.sems.allocated().values()))
        self.nc.all_engine_barrier()

B, N, H, D = 4, 16384, 8, 128
KS, STRIDE = 32, 16
M = (N - KS) // STRIDE + 1      # 1023
NCORES = 8
F = H * D                        # 1024 free elems (head, d)
P = 128                          # partitions / tokens per chunk
MCORE = 512                      # outputs per core
NCH = 64                         # full 128-token chunks per core
T_SLAB = NCH * P + STRIDE        # 8208 tokens (16 tail tokens)
NQ = 4                           # m-quarters of 128 outputs

_prog_cache = {}


def _split_multi_waits(nc):
    """TRN2 instructions carry one sync-wait slot; Tile sometimes attaches
    more (slot-recycle + DMA-lane).  Hoist extras onto single-wait nops
    inserted just before the instruction on the same engine queue —
    identical semantics, accepted by walrus codegen."""
    import bass_rust
    for func in nc.m.functions:
        for bb in func.blocks:
            insts = list(bb.instructions)
            out, changed = [], False
            for inst in insts:
                si = inst.sync_info
                if si is not None and len(si.on_wait) > 1:
                    waits = list(si.on_wait)
                    for k, w in enumerate(waits[:-1]):
                        nop = mybir.InstNoOp(name=f"{inst.name}-ws{k}")
                        nop.engine = inst.engine
                        nop.sync_info = bass_rust.SyncInfo(
                            on_wait=[w], on_update=[])
                        out.append(nop)
                    inst.sync_info = bass_rust.SyncInfo(
                        on_wait=[waits[-1]], on_update=list(si.on_update))
                    changed = True
                out.append(inst)
            if changed:
                bb.instructions = out


def _build_program(reps=1):
    """Build the SPMD Bass/Tile program (identical for all 8 cores).

    reps>1 repeats the whole pipeline inside one NEFF (benchmark use:
    slope of wall time vs reps isolates device execution time from the
    dispatch round trip)."""
    nc = bass.Bass("TRN2", target_bir_lowering=False, debug=False,
                   num_devices=NCORES)
    x_d = nc.dram_tensor("x", [T_SLAB, F], mybir.dt.float8e3,
                         kind="ExternalInput").ap()
    wb_d = nc.dram_tensor("wb", [P, 9], mybir.dt.bfloat16,
                          kind="ExternalInput").ap()
    b2_d = nc.dram_tensor("b2", [2, P], mybir.dt.bfloat16,
                          kind="ExternalInput").ap()
    on_d = nc.dram_tensor("on2", [2, MCORE], mybir.dt.bfloat16,
                          kind="ExternalInput").ap()
    y_d = nc.dram_tensor("y", [H, P, MCORE], mybir.dt.bfloat16,
                         kind="ExternalOutput").ap()
    y_v = y_d.rearrange("q p m -> p q m")

    # chunk-group sizes: small first group so compute starts early
    sizes = [2] + [4] * 15 + [2]
    assert sum(sizes) == NCH

    with _SplitDrainTileContext(nc) as tc, ExitStack() as ctx:
        const_pool = ctx.enter_context(tc.tile_pool(name="const", bufs=1))
        chunk_pool = ctx.enter_context(tc.tile_pool(name="chunks", bufs=5))
        tail_pool = ctx.enter_context(tc.tile_pool(name="tails", bufs=2))
        out_pool = ctx.enter_context(tc.tile_pool(name="out", bufs=2))
        psum_pool = ctx.enter_context(
            tc.tile_pool(name="psum", bufs=6, space="PSUM"))

        wb = const_pool.tile([P, 9], mybir.dt.bfloat16)
        nc.gpsimd.dma_start(out=wb[:], in_=wb_d)
        b2 = const_pool.tile([2, P], mybir.dt.bfloat16)
        nc.gpsimd.dma_start(out=b2[:], in_=b2_d)
        on2 = const_pool.tile([2, MCORE], mybir.dt.bfloat16)
        nc.gpsimd.dma_start(out=on2[:], in_=on_d)

        for _rep in range(reps):
            outsb = out_pool.tile([P, H * MCORE], mybir.dt.bfloat16,
                                  name="osb", tag="osb")
            # 16 tail tokens, loaded up-front (tiny)
            tl = tail_pool.tile([16, F], mybir.dt.float8e3,
                                name="tl", tag="tl")
            nc.gpsimd.dma_start(out=tl[:], in_=x_d[ds(P * NCH, 16)])

            ps = {}          # (Q, j) -> psum tile: quarter Q, heads 4j..4j+3

            def init_quarter(Q):
                for j in range(2):
                    ps[(Q, j)] = psum_pool.tile(
                        [P, 512], mybir.dt.float32, name="ps", tag="ps")
                    # zero + bias broadcast via rank-2 ones matmul
                    nc.tensor.matmul(
                        ps[(Q, j)][:], lhsT=b2[:], rhs=on2[:],
                        start=True, stop=False, skip_group_check=True)

            def evict_quarter(Q):
                for j in range(2):
                    eng = nc.vector if j == 0 else nc.scalar
                    src = ps[(Q, j)][:].rearrange("p (i m) -> p i m", i=4)
                    dst = outsb[:].rearrange(
                        "p (q m) -> p q m", q=H)[:, ds(4 * j, 4),
                                                 ds(Q * P, P)]
                    if j == 0:
                        eng.tensor_copy(dst, src)
                    else:
                        eng.copy(dst, src)

            def store_half(hh):
                nc.scalar.dma_start(
                    out=y_v[:, :, ds(256 * hh, 256)],
                    in_=outsb[:].rearrange(
                        "p (q m) -> p q m", q=H)[:, :, ds(256 * hh, 256)])

            def a_cols(grp, ci, Q):
                for q in range(H):
                    nc.tensor.matmul(
                        ps[(Q, q // 4)][:, ds((q % 4) * P, 8)],
                        lhsT=grp[:, ds(ci * F + q * P, P)],
                        rhs=wb[:, 1:9],
                        start=False, stop=False, skip_group_check=True)

            def process(grp, ci, c):
                Q, r = c // 16, (c % 16) * 8
                if c == 0:
                    init_quarter(0)
                    a_cols(grp, ci, 0)
                elif r == 0:
                    # boundary: B-column closes quarter Q-1
                    for q in range(H):
                        nc.tensor.matmul(
                            ps[(Q - 1, q // 4)][:, ds((q % 4) * P + 127, 1)],
                            lhsT=grp[:, ds(ci * F + q * P, P)],
                            rhs=wb[:, 0:1],
                            start=False, stop=True, skip_group_check=True)
                    evict_quarter(Q - 1)
                    if Q == 2:
                        store_half(0)
                    init_quarter(Q)
                    a_cols(grp, ci, Q)
                else:
                    for q in range(H):
                        nc.tensor.matmul(
                            ps[(Q, q // 4)][:, ds((q % 4) * P + r - 1, 9)],
                            lhsT=grp[:, ds(ci * F + q * P, P)],
                            rhs=wb[:, 0:9],
                            start=False, stop=False, skip_group_check=True)

            g = 0
            for pc in sizes:
                grp = chunk_pool.tile([P, pc * F], mybir.dt.float8e3,
                                      name="grp", tag="chunk")
                if pc > 1:
                    nc.sync.dma_start(
                        out=grp[:].rearrange("p (c f) -> p c f", c=pc),
                        in_=x_d[ds(P * g, P * pc)].rearrange(
                            "(c p) f -> p c f", p=P))
                else:
                    nc.sync.dma_start(out=grp[:], in_=x_d[ds(P * g, P)])
                for ci in range(pc):
                    process(grp, ci, g + ci)
                g += pc

            # final B-column from the 16 tail tokens closes quarter 3
            for q in range(H):
                nc.tensor.matmul(
                    ps[(3, q // 4)][:, ds((q % 4) * P + 127, 1)],
                    lhsT=tl[:, ds(q * P, P)],
                    rhs=wb[0:16, 0:1],
                    start=False, stop=True, skip_group_check=True)
            evict_quarter(3)
            store_half(1)
    _split_multi_waits(nc)
    return nc


def _get_program(reps=1):
    if reps not in _prog_cache:
        _prog_cache[reps] = _build_program(reps)
    return _prog_cache[reps]


def _host_prep(x, weight, pe):
    """Build per-core input maps (band matrix, bias rows, fp8 slabs)."""
    x = np.asarray(x)
    w32 = np.asarray(weight, dtype=np.float32)
    pe32 = np.asarray(pe, dtype=np.float32)

    u = np.arange(P)[:, None] // STRIDE
    s = np.arange(P)[:, None] % STRIDE
    p = np.arange(9)[None, :]
    wband = (np.where(u == p - 1, w32[s], 0.0)
             + np.where(u == p, w32[s + STRIDE], 0.0)) / KS
    wband = wband.astype(BF16)

    bias_d = ((w32[:, None].astype(np.float64) * pe32).sum(0) / KS
              ).astype(np.float32)
    bias_hi = bias_d.astype(BF16)
    bias_lo = (bias_d - bias_hi.astype(np.float32)).astype(BF16)
    b2 = np.stack([bias_hi, bias_lo])            # [2, 128]
    on2 = np.ones((2, MCORE), dtype=BF16)

    in_maps = []
    for c in range(NCORES):
        b, hh = c // 2, c % 2
        base = 8192 * hh
        t_valid = min(N - base, T_SLAB)
        slab = np.zeros((T_SLAB, F), dtype=E3M4)
        slab[:t_valid] = x[b, base:base + t_valid].reshape(
            t_valid, F).astype(E3M4)
        in_maps.append({"x": slab, "wb": wband, "b2": b2, "on2": on2})
    return in_maps


def _assemble(results, dtype):
    y = np.empty((B, M, H, D), dtype=np.float32)
    for c in range(NCORES):
        b, hh = c // 2, c % 2
        rows = 512 if hh == 0 else M - 512
        part = results[c]["y"].astype(np.float32)   # [H, D, MCORE]
        y[b, 512 * hh:512 * hh + rows] = part.transpose(2, 0, 1)[:rows]
    return y.astype(dtype, copy=False)


def kernel(x, weight, pe):
    nc = _get_program()
    in_maps = _host_prep(x, weight, pe)
    res = run_bass_kernel_spmd(nc, in_maps, list(range(NCORES)))
    return _assemble(res.results, np.asarray(x).dtype)


# revision 28
# speedup vs baseline: 1.8742x; 1.0890x over previous
"""Strided depthwise-conv ("CompressKV") kernel for 8 Trainium2 NeuronCores.

y[b,m,h,d] = (sum_k x[b, 16*m+k, h, d] * w[k] + sum_k pe[k,d]*w[k]) / 32
B=4, N=16384, H=8, D=128, K=32, STRIDE=16, M=1023.

Strategy
--------
Shard: core <-> (batch b, sequence half). Each core owns the token slab
x[b, 8192*hh : 8192*hh+8208] (zero-padded past N), all 8 heads.

Compute: x is the STATIONARY matmul operand. Since kernel_size = 2*stride,
each 16-token block j feeds exactly two outputs:

    y[m] = sum_s x_r[m,   s]*w[s]      (A-term, s<16)
         + sum_s x_r[m+1, s]*w[16+s]   (B-term)

so a 128-token chunk c (blocks 8c..8c+7) contributes to the 9 outputs
m = 8c-1 .. 8c+7 with a CONSTANT [128, 9] banded weight matrix wband:
wband[16u+s, p] = w[s]/32 if u==p-1 else w[16+s]/32 if u==p else 0.

Per (chunk, head): one matmul with lhsT = x_chunk[:, head] (fp8 e3m4,
128x128) and rhs = wband (bf16, 128x9), accumulated into PSUM holding
y^T[d, m] for that head. The moving operand is 9 columns wide, so PE time
is negligible; x rides the zero-cost weight-load path. x is cast to
fp8e3m4 on the host (halves DMA vs bf16; measured end-to-end rel err
9.5e-3 vs the 2e-2 budget). The pe-bias enters via a rank-2 ones matmul
(bf16 hi/lo split) that also zero-initializes each PSUM region.

PSUM: one 2KB bank = [128, 512] fp32 holds one m-quarter (128 outputs)
for 4 heads. Quarters evict (fp32->bf16 copy on DVE/Act) as soon as the
next quarter's boundary B-column lands, and each 256-output half stores
with a single DMA (512B-contiguous rows).
"""

import numpy as np
import ml_dtypes
from contextlib import ExitStack

import concourse.bass as bass
import concourse.mybir as mybir
import concourse.tile as tile
from concourse.bass import ds, ts
from concourse.bass_utils import run_bass_kernel_spmd

BF16 = ml_dtypes.bfloat16
E3M4 = ml_dtypes.float8_e3m4


class _SplitDrainTileContext(tile.TileContext):
    """TileContext whose kernel-tail drain carries at most one sem wait.

    TRN2 instructions have a single sync-wait slot; the stock tail drain
    aggregates one wait per logical processor (14 here), which walrus
    rejects.  Move the extras onto dedicated single-wait nops on the same
    (sync) queue ahead of the all-engine barrier — identical semantics.
    """

    def _drain_and_barrier(self, tick_clock, wait_clock):
        import bass_rust
        from concourse.vector_clock import ScopedClock

        drain_inst = self.nc.sync.drain()
        wait_clock.add_sem_waits(
            drain_inst.ins, ScopedClock({None: tick_clock.global_clock}))
        si = drain_inst.ins.sync_info
        if si is not None and len(si.on_wait) > 1:
            waits = list(si.on_wait)
            drain_inst.ins.sync_info = bass_rust.SyncInfo(
                on_wait=[waits[0]], on_update=list(si.on_update))
            for w in waits[1:]:
                nop = self.nc.sync.nop(hint="drain_split", nofuse=True)
                nop.ins.sync_info = bass_rust.SyncInfo(
                    on_wait=[w], on_update=[])

        self.nc.all_engine_barrier()
        assert self.sems is not None
        popped = self.nc._tile_sem_poison_stack.pop()
        assert popped is self._sem_poison
        self.nc.clear_and_free_semaphores(
            list(self.sems.allocated().values()))
        self.nc.all_engine_barrier()

B, N, H, D = 4, 16384, 8, 128
KS, STRIDE = 32, 16
M = (N - KS) // STRIDE + 1      # 1023
NCORES = 8
F = H * D                        # 1024 free elems (head, d)
P = 128                          # partitions / tokens per chunk
MCORE = 512                      # outputs per core
NCH = 64                         # full 128-token chunks per core
T_SLAB = NCH * P + STRIDE        # 8208 tokens (16 tail tokens)
NQ = 4                           # m-quarters of 128 outputs

_prog_cache = {}


def _split_multi_waits(nc):
    """TRN2 instructions carry one sync-wait slot; Tile sometimes attaches
    more (slot-recycle + DMA-lane).  Hoist extras onto single-wait nops
    inserted just before the instruction on the same engine queue —
    identical semantics, accepted by walrus codegen."""
    import bass_rust
    for func in nc.m.functions:
        for bb in func.blocks:
            insts = list(bb.instructions)
            out, changed = [], False
            for inst in insts:
                si = inst.sync_info
                if si is not None and len(si.on_wait) > 1:
                    waits = list(si.on_wait)
                    for k, w in enumerate(waits[:-1]):
                        nop = mybir.InstNoOp(name=f"{inst.name}-ws{k}")
                        nop.engine = inst.engine
                        nop.sync_info = bass_rust.SyncInfo(
                            on_wait=[w], on_update=[])
                        out.append(nop)
                    inst.sync_info = bass_rust.SyncInfo(
                        on_wait=[waits[-1]], on_update=list(si.on_update))
                    changed = True
                out.append(inst)
            if changed:
                bb.instructions = out


def _build_program(reps=1):
    """Build the SPMD Bass/Tile program (identical for all 8 cores).

    reps>1 repeats the whole pipeline inside one NEFF (benchmark use:
    slope of wall time vs reps isolates device execution time from the
    dispatch round trip)."""
    nc = bass.Bass("TRN2", target_bir_lowering=False, debug=False,
                   num_devices=NCORES)
    x_d = nc.dram_tensor("x", [T_SLAB, F], mybir.dt.float8e3,
                         kind="ExternalInput").ap()
    wb_d = nc.dram_tensor("wb", [P, 9], mybir.dt.bfloat16,
                          kind="ExternalInput").ap()
    cs_d = nc.dram_tensor("cst", [2, P + MCORE], mybir.dt.bfloat16,
                          kind="ExternalInput").ap()
    y_d = nc.dram_tensor("y", [H, P, MCORE], mybir.dt.bfloat16,
                         kind="ExternalOutput").ap()
    y_v = y_d.rearrange("q p m -> p q m")

    # chunk-group sizes: small first group so compute starts early
    sizes = [2] + [4] * 15 + [2]
    assert sum(sizes) == NCH

    with _SplitDrainTileContext(nc) as tc, ExitStack() as ctx:
        const_pool = ctx.enter_context(tc.tile_pool(name="const", bufs=1))
        chunk_pool = ctx.enter_context(tc.tile_pool(name="chunks", bufs=10))
        tail_pool = ctx.enter_context(tc.tile_pool(name="tails", bufs=2))
        out_pool = ctx.enter_context(tc.tile_pool(name="out", bufs=2))
        psum_pool = ctx.enter_context(
            tc.tile_pool(name="psum", bufs=6, space="PSUM"))

        wb = const_pool.tile([P, 9], mybir.dt.bfloat16)
        nc.gpsimd.dma_start(out=wb[:], in_=wb_d)
        cst = const_pool.tile([2, P + MCORE], mybir.dt.bfloat16)
        nc.gpsimd.dma_start(out=cst[:], in_=cs_d)
        b2 = cst[:, 0:P]
        on2 = cst[:, ds(P, MCORE)]

        for _rep in range(reps):
            outsb = out_pool.tile([P, H * MCORE], mybir.dt.bfloat16,
                                  name="osb", tag="osb")
            # 16 tail tokens, loaded up-front (tiny)
            tl = tail_pool.tile([16, F], mybir.dt.float8e3,
                                name="tl", tag="tl")
            nc.gpsimd.dma_start(out=tl[:], in_=x_d[ds(P * NCH, 16)])

            ps = {}          # (Q, j) -> psum tile: quarter Q, heads 4j..4j+3

            def init_quarter(Q):
                for j in range(2):
                    ps[(Q, j)] = psum_pool.tile(
                        [P, 512], mybir.dt.float32, name="ps", tag="ps")
                    # zero + bias broadcast via rank-2 ones matmul
                    nc.tensor.matmul(
                        ps[(Q, j)][:], lhsT=b2, rhs=on2,
                        start=True, stop=False, skip_group_check=True)

            def evict_tile(Q, j):
                src = ps[(Q, j)][:].rearrange("p (i m) -> p i m", i=4)
                dst = outsb[:].rearrange(
                    "p (q m) -> p q m", q=H)[:, ds(4 * j, 4), ds(Q * P, P)]
                if j == 0:
                    nc.vector.tensor_copy(dst, src)
                else:
                    nc.scalar.copy(dst, src)

            def evict_quarter(Q):
                for j in range(2):
                    evict_tile(Q, j)

            def store_half(hh, eng):
                eng.dma_start(
                    out=y_v[:, :, ds(256 * hh, 256)],
                    in_=outsb[:].rearrange(
                        "p (q m) -> p q m", q=H)[:, :, ds(256 * hh, 256)])

            def a_cols(grp, ci, Q):
                for q in range(H):
                    nc.tensor.matmul(
                        ps[(Q, q // 4)][:, ds((q % 4) * P, 8)],
                        lhsT=grp[:, ds(ci * F + q * P, P)],
                        rhs=wb[:, 1:9],
                        start=False, stop=False, skip_group_check=True)

            def process(grp, ci, c):
                Q, r = c // 16, (c % 16) * 8
                if c == 0:
                    init_quarter(0)
                    a_cols(grp, ci, 0)
                elif r == 0:
                    # boundary: B-column closes quarter Q-1
                    for q in range(H):
                        nc.tensor.matmul(
                            ps[(Q - 1, q // 4)][:, ds((q % 4) * P + 127, 1)],
                            lhsT=grp[:, ds(ci * F + q * P, P)],
                            rhs=wb[:, 0:1],
                            start=False, stop=True, skip_group_check=True)
                    evict_quarter(Q - 1)
                    if Q == 2:
                        store_half(0, nc.scalar)
                    init_quarter(Q)
                    a_cols(grp, ci, Q)
                else:
                    for q in range(H):
                        nc.tensor.matmul(
                            ps[(Q, q // 4)][:, ds((q % 4) * P + r - 1, 9)],
                            lhsT=grp[:, ds(ci * F + q * P, P)],
                            rhs=wb[:, 0:9],
                            start=False, stop=False, skip_group_check=True)

            def evict_slice(Q, j, lo, n, eng, fn):
                src = ps[(Q, j)][:].rearrange(
                    "p (i m) -> p i m", i=4)[:, :, ds(lo, n)]
                dst = outsb[:].rearrange(
                    "p (q m) -> p q m", q=H)[:, ds(4 * j, 4),
                                             ds(Q * P + lo, n)]
                fn(dst, src)

            g = 0
            for pc in sizes[:-1]:
                grp = chunk_pool.tile([P, pc * F], mybir.dt.float8e3,
                                      name="grp", tag="chunk")
                nc.sync.dma_start(
                    out=grp[:].rearrange("p (c f) -> p c f", c=pc),
                    in_=x_d[ds(P * g, P * pc)].rearrange(
                        "(c p) f -> p c f", p=P))
                for ci in range(pc):
                    process(grp, ci, g + ci)
                    if g + ci == 60:
                        # m 384..479 of quarter 3 are final once chunk 60's
                        # B-column lands; pre-evict so only a 32-col sliver
                        # remains on the tail critical path
                        evict_slice(3, 0, 0, 96, nc.vector,
                                    nc.vector.tensor_copy)
                        evict_slice(3, 1, 0, 96, nc.scalar, nc.scalar.copy)
                g += pc

            # Last group: split load/evict/store by head-half so the
            # heads-0-3 chain pipelines ahead of the heads-4-7 chain.
            pc = sizes[-1]
            HF = F // 2
            halves = []
            dram = x_d[ds(P * g, P * pc)].rearrange("(c p) f -> p c f", p=P)
            for jh in range(2):
                gh = chunk_pool.tile([P, pc * HF], mybir.dt.float8e3,
                                     name="gh", tag="chunkh")
                nc.sync.dma_start(
                    out=gh[:].rearrange("p (c f) -> p c f", c=pc),
                    in_=dram[:, :, ds(jh * HF, HF)])
                halves.append(gh)
            for j, eng in ((0, nc.sync), (1, nc.scalar)):
                for ci in range(pc):
                    c = g + ci
                    Q, r = c // 16, (c % 16) * 8
                    assert r != 0 and Q == 3
                    for q in range(4 * j, 4 * j + 4):
                        nc.tensor.matmul(
                            ps[(3, j)][:, ds((q % 4) * P + r - 1, 9)],
                            lhsT=halves[j][:, ds(ci * HF + (q % 4) * P, P)],
                            rhs=wb[:, 0:9],
                            start=False, stop=False, skip_group_check=True)
                # final B-column from the 16 tail tokens closes quarter 3
                for q in range(4 * j, 4 * j + 4):
                    nc.tensor.matmul(
                        ps[(3, j)][:, ds((q % 4) * P + 127, 1)],
                        lhsT=tl[:, ds(q * P, P)],
                        rhs=wb[0:16, 0:1],
                        start=False, stop=True, skip_group_check=True)
                if j == 0:
                    evict_slice(3, 0, 96, 32, nc.vector,
                                nc.vector.tensor_copy)
                else:
                    evict_slice(3, 1, 96, 32, nc.scalar, nc.scalar.copy)
                eng.dma_start(
                    out=y_v[:, ds(4 * j, 4), ds(256, 256)],
                    in_=outsb[:].rearrange(
                        "p (q m) -> p q m", q=H)[:, ds(4 * j, 4),
                                                 ds(256, 256)])
    _split_multi_waits(nc)
    return nc


def _get_program(reps=1):
    if reps not in _prog_cache:
        _prog_cache[reps] = _build_program(reps)
    return _prog_cache[reps]


def _host_prep(x, weight, pe):
    """Build per-core input maps (band matrix, bias rows, fp8 slabs)."""
    x = np.asarray(x)
    w32 = np.asarray(weight, dtype=np.float32)
    pe32 = np.asarray(pe, dtype=np.float32)

    u = np.arange(P)[:, None] // STRIDE
    s = np.arange(P)[:, None] % STRIDE
    p = np.arange(9)[None, :]
    wband = (np.where(u == p - 1, w32[s], 0.0)
             + np.where(u == p, w32[s + STRIDE], 0.0)) / KS
    wband = wband.astype(BF16)

    bias_d = ((w32[:, None].astype(np.float64) * pe32).sum(0) / KS
              ).astype(np.float32)
    bias_hi = bias_d.astype(BF16)
    bias_lo = (bias_d - bias_hi.astype(np.float32)).astype(BF16)
    cst = np.ones((2, P + MCORE), dtype=BF16)
    cst[0, :P] = bias_hi
    cst[1, :P] = bias_lo

    in_maps = []
    for c in range(NCORES):
        b, hh = c // 2, c % 2
        base = 8192 * hh
        t_valid = min(N - base, T_SLAB)
        slab = np.zeros((T_SLAB, F), dtype=E3M4)
        slab[:t_valid] = x[b, base:base + t_valid].reshape(
            t_valid, F).astype(E3M4)
        in_maps.append({"x": slab, "wb": wband, "cst": cst})
    return in_maps


def _assemble(results, dtype):
    y = np.empty((B, M, H, D), dtype=np.float32)
    for c in range(NCORES):
        b, hh = c // 2, c % 2
        rows = 512 if hh == 0 else M - 512
        part = results[c]["y"].astype(np.float32)   # [H, D, MCORE]
        y[b, 512 * hh:512 * hh + rows] = part.transpose(2, 0, 1)[:rows]
    return y.astype(dtype, copy=False)


def kernel(x, weight, pe):
    nc = _get_program()
    in_maps = _host_prep(x, weight, pe)
    res = run_bass_kernel_spmd(nc, in_maps, list(range(NCORES)))
    return _assemble(res.results, np.asarray(x).dtype)
